# revision 49
# baseline (speedup 1.0000x reference)
"""Trainium2 Bass kernel for nn_AdaConvNeXt (moe_routing).

Data-parallel over batch (16 images/core).  Major design points vs the
previous version:
  - All FFN matmuls in fp8e4 with DoubleRow (K=256 per instruction):
    W1 (2 MMs per fg/half), W2f (6 MMs), W2q (2 MMs).
  - Routing masks are folded into the LayerNorm scale rows on the stats
    partitions (im = istd*m, nm = -mu*istd*m), so z1 = y*im1+nm1 and
    z2 = y*im2+nm2 are the *pre-masked* branch inputs.  Because gelu(0)=0
    and the biases are zero (fast path), both branch outputs accumulate
    into a single PSUM group: s12 = W2f^T gelu(W1^T z1) + W2q^T z2.
    Nonzero biases are handled by extra K=1 rank-1 matmuls (slow path).
  - Depthwise 7x7 conv split across engines with zero-padded halo tiles
    (row stride 48): PE gets fp8 DoubleRow tap *pairs* ((dy,dy+1), same
    dx) via hand-built overlapping access patterns; DVE gets fused
    STT multiply-adds for even-dx taps; ACT computes shifted products
    (alignment-immune) that GpSimd accumulates.
  - LN stats via ones-matmuls (both halves share one PSUM bank via
    tile_position), batched row math over 4 images.
"""

import os
import numpy as np
import ml_dtypes

import concourse.bass as bass
import concourse.bacc as bacc
import concourse.mybir as mybir
import concourse.tile as tile
from concourse.bass_utils import run_bass_kernel_spmd

BF16 = mybir.dt.bfloat16
FP8 = mybir.dt.float8e4
F32 = mybir.dt.float32
ADD = mybir.AluOpType.add
SUB = mybir.AluOpType.subtract
MULT = mybir.AluOpType.mult
AF = mybir.ActivationFunctionType
DR = mybir.MatmulPerfMode.DoubleRow

N_CORES = 8
B, C, H, W = 128, 384, 28, 28
N = H * W          # 784
BL = B // N_CORES  # 16 images per core
NG = C // 128      # 3 channel groups
FG = (4 * C) // 128  # 12 ffn groups
HALF = N // 2      # 392 = one PSUM bank of f32
EPS = 1e-6
STAT_BLK = 4       # images per batched-stats block (partitions 0,32,64,96)

# halo layout: row r = R0 + (h+dy), col = C0 + (w+dx), row stride HS
HS = 48
HR = 34
R0, C0 = 3, 4
NSLOT = 2          # x halo tile slots (double buffering)

# --- tap split (tunable) -----------------------------------------------------
# PE: dy-pairs (-3,-2),(-1,0),(1,2) per listed dx column
PE_PAIRS = [((a, dx), (a + 1, dx)) for dx in (-3, -1, 1, 3, 0, -2)
            for a in (-3, -1, 1)]
PE_SINGLES = []
# elementwise leftovers: dy=3 row + column dx=2
DVE_TAPS = [(3, 0), (3, -2), (-3, 2), (-2, 2), (-1, 2), (0, 2)]
_ASSIGNED = {t for p in PE_PAIRS for t in p} | set(PE_SINGLES) | set(DVE_TAPS)
ACT_TAPS = [(dy, dx) for dy in range(-3, 4) for dx in range(-3, 4)
            if (dy, dx) not in _ASSIGNED]
NPAIR = len(PE_PAIRS)
NSING = len(PE_SINGLES)
NDVE = len(DVE_TAPS)
NACT = len(ACT_TAPS)
assert 2 * NPAIR + NSING + NDVE + NACT == 49
DVE_INIT = (3, 0)  # via full-window halo product (init covers full acc)


def build_bass(BL_, slow_bias):
    nc = bacc.Bacc(None, target_bir_lowering=False, debug=False)

    x_d = nc.declare_dram_parameter("x", [BL_, C, H, W], F32, isOutput=False)
    # per image: [branch, {C*m, m}, N]
    mrows_d = nc.declare_dram_parameter("mrows", [BL_, 2, 2, N], BF16,
                                        isOutput=False)
    diag2_d = nc.declare_dram_parameter("diag2", [128, NG, NPAIR, 2, 128], FP8,
                                        isOutput=False)
    diag1_d = nc.declare_dram_parameter("diag1", [128, NG, max(NSING, 1), 128],
                                        FP8, isOutput=False)
    w1t_d = nc.declare_dram_parameter("w1t", [128, NG, FG, 128], FP8, isOutput=False)
    w2ft_d = nc.declare_dram_parameter("w2ft", [128, FG, NG, 128], FP8, isOutput=False)
    w2qt_d = nc.declare_dram_parameter("w2qt", [128, NG, NG, 128], FP8, isOutput=False)
    dwtap_d = nc.declare_dram_parameter("dwtap", [128, NG, NDVE + NACT], F32,
                                        isOutput=False)
    dwb_d = nc.declare_dram_parameter("dwb", [128, NG], F32, isOutput=False)
    if slow_bias:
        # c1*S1 per fg block; (c1out*S2, c2*S2) per og block
        c1t_d = nc.declare_dram_parameter("c1t", [1, FG, 128], BF16, isOutput=False)
        ct_d = nc.declare_dram_parameter("ct", [1, NG, 2, 128], BF16, isOutput=False)
    out_d = nc.declare_dram_parameter("out", [BL_, C, H, W], F32, isOutput=True)

    from contextlib import ExitStack
    with ExitStack() as es:
        tc = es.enter_context(tile.TileContext(nc))
        pool = lambda name, bufs, **kw: es.enter_context(
            tc.tile_pool(name=name, bufs=bufs, **kw))
        cpool = pool("consts", 1)
        acc_pool = pool("acc", 2)
        tmp_pool = pool("tmpp", 2)
        y_pool = pool("ybuf", STAT_BLK + 2)
        ysq_pool = pool("ysq", 1)
        t_pool = pool("tbuf", 2)
        z_pool = pool("zbuf", 2)
        g_pool = pool("gbuf", 2)
        bc_pool = pool("bcast", 2)
        o_pool = pool("obuf", 2)
        rows_pool = pool("rows", 1)
        dram_pool = pool("dscratch", 2 * STAT_BLK, space=bass.MemorySpace.DRAM)
        pyc_pool = pool("pyc", 2, space=bass.MemorySpace.PSUM)
        ph_pool = pool("ph", 2, space=bass.MemorySpace.PSUM)
        ps_pool = pool("ps", 2, space=bass.MemorySpace.PSUM)

        # ---- constants ----
        diag2_sb = cpool.tile([128, NG, NPAIR, 2, 128], FP8)
        nc.sync.dma_start(diag2_sb[:], diag2_d[:])
        diag1_sb = cpool.tile([128, NG, max(NSING, 1), 128], FP8)
        nc.sync.dma_start(diag1_sb[:], diag1_d[:])
        w1t_sb = cpool.tile([128, NG, FG, 128], FP8)
        nc.sync.dma_start(w1t_sb[:], w1t_d[:])
        w2ft_sb = cpool.tile([128, FG, NG, 128], FP8)
        nc.sync.dma_start(w2ft_sb[:], w2ft_d[:])
        w2qt_sb = cpool.tile([128, NG, NG, 128], FP8)
        nc.sync.dma_start(w2qt_sb[:], w2qt_d[:])
        dwtap_sb = cpool.tile([128, NG, NDVE + NACT], F32)
        nc.sync.dma_start(dwtap_sb[:], dwtap_d[:])
        dwb_sb = cpool.tile([128, NG], F32)
        nc.sync.dma_start(dwb_sb[:], dwb_d[:])
        if slow_bias:
            c1t_sb = cpool.tile([1, FG, 128], BF16)
            nc.sync.dma_start(c1t_sb[:], c1t_d[:])
            ct_sb = cpool.tile([1, NG, 2, 128], BF16)
            nc.sync.dma_start(ct_sb[:], ct_d[:])

        ones_col = cpool.tile([128, 1], BF16)
        nc.vector.memset(ones_col[:], 1.0)
        eps_col = cpool.tile([97, 1], F32)
        nc.vector.memset(eps_col[:], float(C) * float(C) * EPS)

        # persistent slotted halo tiles, borders zeroed once
        xf8 = cpool.tile([128, NSLOT, NG, HR, HS], FP8)
        xbf = cpool.tile([128, NSLOT, NG, HR, HS], BF16)
        for s in range(NSLOT):
            nc.gpsimd.memset(xf8[:, s], 0.0)
            nc.vector.memset(xbf[:, s], 0.0)

        def bcast3(tile_ap, n_inner):
            """[128, N]-tile AP broadcast to [128, NG, n_inner] via step-0 dim."""
            a = tile_ap
            return bass.AP(a.tensor, a.offset,
                           ap=[list(a.ap[0]), [0, NG], [1, n_inner]])

        def flat2(a, n_inner):
            """Contiguous free dims viewed as [2, n_inner]."""
            return bass.AP(a.tensor, a.offset,
                           ap=[list(a.ap[0]), [n_inner, 2], [1, n_inner]])

        def halo_win(xt, s, g, dy, dx, rows=H, cols=W, r_off=0):
            """AP over halo tile: [rows, cols] window shifted by (dy, dx)."""
            return xt[:, s, g,
                      R0 + r_off + dy: R0 + r_off + dy + rows,
                      C0 + dx: C0 + dx + cols]

        def pair_rhs(xt, s, g, dy0, dx, lam):
            """Hand-built overlapping AP [2, 14, 28] for DR tap pair."""
            base = xt[:, s, g]
            off = base.offset + (R0 + 14 * lam + dy0) * HS + (C0 + dx)
            return bass.AP(base.tensor, off,
                           ap=[list(base.ap[0]), [HS, 2], [HS, 14], [1, 28]])

        y_tiles = {}
        stat_dr = {}
        rows_t = {}
        mr1_t = {}

        def ensure_rows(blk):
            srow = rows_pool.tile([97, N], F32, tag="srow")
            qrow = rows_pool.tile([97, N], F32, tag="qrow")
            mrow = rows_pool.tile([97, 2, 2, N], BF16, tag="mrow")
            imgs = list(range(blk * STAT_BLK, min((blk + 1) * STAT_BLK, BL_)))
            for ii, img in enumerate(imgs):
                nc.sync.dma_start(out=mrow[32 * ii:32 * ii + 1],
                                  in_=mrows_d[img])
            rows_t[blk] = (srow, qrow, mrow, imgs)

        def conv_phase(img):
            blk, ii = divmod(img, STAT_BLK)
            if ii == 0:
                ensure_rows(blk)
            srow, qrow, _mrow, _imgs = rows_t[blk]
            s = img % NSLOT
            for g in range(NG):
                nc.gpsimd.dma_start(
                    out=xbf[:, s, g, R0:R0 + H, C0:C0 + W],
                    in_=x_d[img, g * 128:(g + 1) * 128])
                nc.gpsimd.dma_start(
                    out=xf8[:, s, g, R0:R0 + H, C0:C0 + W],
                    in_=x_d[img, g * 128:(g + 1) * 128])
                # residual prefill; branch outputs DMA-accumulate later
                nc.sync.dma_start(
                    out=out_d[img, g * 128:(g + 1) * 128],
                    in_=x_d[img, g * 128:(g + 1) * 128])

            # conv: elementwise part, two parallel accumulator chains
            acc = acc_pool.tile([128, NG, H, W], BF16, tag="accA")
            accB = acc_pool.tile([128, NG, H, W], BF16, tag="accB")
            k_init = DVE_TAPS.index(DVE_INIT)
            dy0, dx0 = DVE_INIT
            for g in range(NG):
                nc.vector.tensor_scalar(
                    out=acc[:, g], in0=halo_win(xbf, s, g, dy0, dx0),
                    scalar1=dwtap_sb[:, g, k_init:k_init + 1],
                    scalar2=dwb_sb[:, g:g + 1], op0=MULT, op1=ADD)
            for k, (dy, dx) in enumerate(DVE_TAPS):
                if (dy, dx) == DVE_INIT:
                    continue
                dtmp = tmp_pool.tile([128, NG, H, W], BF16, tag="dtmp")
                for g in range(NG):
                    nc.vector.tensor_scalar(
                        out=dtmp[:, g], in0=halo_win(xbf, s, g, dy, dx),
                        scalar1=dwtap_sb[:, g, k:k + 1],
                        scalar2=None, op0=MULT)
                nc.vector.tensor_tensor(
                    out=acc[:], in0=acc[:], in1=dtmp[:], op=ADD)
            for j, (dy, dx) in enumerate(ACT_TAPS):
                k = NDVE + j
                if j == 0:
                    tmp = accB  # first ACT product initializes the B chain
                else:
                    tmp = tmp_pool.tile([128, NG, H, W], BF16, tag="atmp")
                for g in range(NG):
                    nc.scalar.activation(
                        tmp[:, g], halo_win(xbf, s, g, dy, dx),
                        AF.Copy, scale=dwtap_sb[:, g, k:k + 1])
                if j > 0:
                    nc.vector.tensor_tensor(
                        out=accB[:], in0=accB[:], in1=tmp[:], op=ADD)

            # conv: PE fp8 DoubleRow pairs + singles
            y_bf = y_pool.tile([128, NG, 2, HALF], BF16)
            y_tiles[img] = y_bf
            for g in range(NG):
                pyc = pyc_pool.tile([128, 2, 512], F32)
                for pi, ((pdy0, pdx), _t1) in enumerate(PE_PAIRS):
                    for lam in range(2):
                        nc.tensor.matmul(
                            pyc[:, lam, 0:HALF],
                            diag2_sb[:, g, pi],
                            pair_rhs(xf8, s, g, pdy0, pdx, lam),
                            start=(pi == 0),
                            stop=(NSING == 0 and pi == NPAIR - 1),
                            perf_mode=DR, skip_group_check=True)
                for si, (dy, dx) in enumerate(PE_SINGLES):
                    for lam in range(2):
                        nc.tensor.matmul(
                            pyc[:, lam, 0:HALF],
                            diag1_sb[:, g, si],
                            halo_win(xf8, s, g, dy, dx, rows=14,
                                     r_off=14 * lam),
                            start=False, stop=(si == NSING - 1),
                            skip_group_check=True)
                # y = psum/S_dw + accA + accB
                nc.vector.scalar_tensor_tensor(
                    out=y_bf[:, g], in0=pyc[:, :, 0:HALF],
                    scalar=1.0 / SD_SCALE,
                    in1=flat2(acc[:, g], HALF),
                    op0=MULT, op1=ADD)
                nc.vector.tensor_tensor(
                    out=y_bf[:, g], in0=y_bf[:, g],
                    in1=flat2(accB[:, g], HALF), op=ADD)

            # LN stats
            ysq = ysq_pool.tile([128, NG, 2, HALF], BF16)
            nc.gpsimd.tensor_tensor(out=ysq[:], in0=y_bf[:], in1=y_bf[:],
                                    op=MULT)
            ps_base = 32 * ii
            for lam in range(2):
                pst = ph_pool.tile([128, HALF], F32, tag="ph")
                for g in range(NG):
                    nc.tensor.matmul(
                        pst[0:1, :], ones_col[:], y_bf[:, g, lam],
                        start=(g == 0), stop=(g == NG - 1),
                        skip_group_check=True)
                for g in range(NG):
                    nc.tensor.matmul(
                        pst[32:33, :], ones_col[:], ysq[:, g, lam],
                        start=(g == 0), stop=(g == NG - 1),
                        tile_position=(0, 32), skip_group_check=True)
                cs = slice(HALF * lam, HALF * lam + HALF)
                if lam == 0:
                    nc.vector.tensor_copy(srow[ps_base:ps_base + 1, cs],
                                          pst[0:1, :])
                    nc.vector.tensor_copy(qrow[ps_base:ps_base + 1, cs],
                                          pst[32:33, :])
                else:
                    nc.scalar.copy(srow[ps_base:ps_base + 1, cs],
                                   pst[0:1, :])
                    nc.scalar.copy(qrow[ps_base:ps_base + 1, cs],
                                   pst[32:33, :])

        def stats_phase(blk):
            srow, qrow, mrow, imgs = rows_t[blk]
            np_ = 32 * (len(imgs) - 1) + 1
            musq = rows_pool.tile([97, N], F32, tag="rw1")
            nc.vector.tensor_tensor(out=musq[:np_], in0=srow[:np_],
                                    in1=srow[:np_], op=MULT)
            veps = rows_pool.tile([97, N], F32, tag="rw2")
            nc.vector.scalar_tensor_tensor(
                out=veps[:np_], in0=qrow[:np_], scalar=float(C),
                in1=musq[:np_], op0=MULT, op1=SUB)
            sd = rows_pool.tile([97, N], F32, tag="rw1")
            nc.scalar.activation(sd[:np_], veps[:np_], AF.Sqrt,
                                 bias=eps_col[:np_])
            istd = rows_pool.tile([97, N], F32, tag="rw3")
            with nc.allow_low_precision(reason="LN istd approx is plenty"):
                nc.vector.reciprocal_approx_fast(out=istd[:np_], in_=sd[:np_])
            mus = rows_pool.tile([97, N], F32, tag="rw2")
            nc.vector.scalar_tensor_tensor(
                out=mus[:np_], in0=srow[:np_], scalar=-1.0,
                in1=istd[:np_], op0=MULT, op1=MULT)
            imr = rows_pool.tile([97, 2, N], BF16, tag="rw4")
            nmr = rows_pool.tile([97, 2, N], BF16, tag="rw5")
            for br in range(2):
                nc.vector.tensor_tensor(
                    out=imr[:np_, br], in0=istd[:np_],
                    in1=mrow[:np_, br, 0], op=MULT)
                nc.vector.tensor_tensor(
                    out=nmr[:np_, br], in0=mus[:np_],
                    in1=mrow[:np_, br, 1], op=MULT)
            # stage rows to DRAM for partition-broadcast
            for ii, img in enumerate(imgs):
                ps_base = 32 * ii
                sc = dram_pool.tile([4, N], BF16, tag="sc", name=f"sc{img}")
                nc.sync.dma_start(out=sc[0:1], in_=imr[ps_base:ps_base + 1, 0])
                nc.sync.dma_start(out=sc[1:2], in_=nmr[ps_base:ps_base + 1, 0])
                nc.sync.dma_start(out=sc[2:3], in_=imr[ps_base:ps_base + 1, 1])
                nc.sync.dma_start(out=sc[3:4], in_=nmr[ps_base:ps_base + 1, 1])
                stat_dr[img] = sc

        def ffn_phase(img):
            y_bf = y_tiles.pop(img)
            sc = stat_dr.pop(img)
            bcs = []
            for r in range(4):
                bt = bc_pool.tile([128, N], BF16, tag=f"bc{r}")
                nc.sync.dma_start(
                    out=bt[:], in_=sc[r:r + 1].partition_broadcast(128))
                bcs.append(bt)
            im1b, nm1b, im2b, nm2b = bcs
            if slow_bias:
                mr1 = bc_pool.tile([1, 2, N], BF16, tag="mr1")
                nc.sync.dma_start(out=mr1[:], in_=mrows_d[img, :, 1])

            z12 = []
            for br, (imb, nmb) in enumerate(((im1b, nm1b), (im2b, nm2b))):
                tb = t_pool.tile([128, NG, 2, HALF], BF16, tag="tb")
                zb = z_pool.tile([128, NG, 2, HALF], FP8, tag=f"z{br}")
                for g in range(NG):
                    nc.vector.tensor_tensor(
                        out=tb[:, g], in0=y_bf[:, g],
                        in1=flat2(imb[:], HALF), op=MULT)
                    nc.vector.tensor_tensor(
                        out=zb[:, g], in0=tb[:, g],
                        in1=flat2(nmb[:], HALF), op=ADD)
                z12.append(zb)
            z1, z2 = z12

            g_sb = g_pool.tile([128, FG, 2, HALF], FP8)
            for fg in range(FG):
                for lam in range(2):
                    ph = ph_pool.tile([128, HALF], F32, tag="ph")
                    nc.tensor.matmul(
                        ph[:], w1t_sb[:, 0:2, fg], z1[:, 0:2, lam],
                        start=True, stop=False, perf_mode=DR)
                    nc.tensor.matmul(
                        ph[:], w1t_sb[:, 2, fg], z1[:, 2, lam],
                        start=False, stop=not slow_bias)
                    if slow_bias:
                        nc.tensor.matmul(
                            ph[:], c1t_sb[:, fg],
                            mr1[:, 0, HALF * lam:HALF * lam + HALF],
                            start=False, stop=True, skip_group_check=True)
                    nc.scalar.activation(
                        g_sb[:, fg, lam], ph[:], AF.Gelu,
                        scale=1.0 / S1_SCALE)

            for og in range(NG):
                osb = o_pool.tile([128, 2, HALF], F32)
                for lam in range(2):
                    ps = ps_pool.tile([128, HALF], F32)
                    for f2 in range(FG // 2):
                        nc.tensor.matmul(
                            ps[:], w2ft_sb[:, 2 * f2:2 * f2 + 2, og],
                            g_sb[:, 2 * f2:2 * f2 + 2, lam],
                            start=(f2 == 0), stop=False, perf_mode=DR)
                    nc.tensor.matmul(
                        ps[:], w2qt_sb[:, 0:2, og], z2[:, 0:2, lam],
                        start=False, stop=False, perf_mode=DR)
                    nc.tensor.matmul(
                        ps[:], w2qt_sb[:, 2, og], z2[:, 2, lam],
                        start=False, stop=not slow_bias)
                    if slow_bias:
                        nc.tensor.matmul(
                            ps[:], ct_sb[:, og, 0],
                            mr1[:, 0, HALF * lam:HALF * lam + HALF],
                            start=False, stop=False, skip_group_check=True)
                        nc.tensor.matmul(
                            ps[:], ct_sb[:, og, 1],
                            mr1[:, 1, HALF * lam:HALF * lam + HALF],
                            start=False, stop=True, skip_group_check=True)
                    nc.scalar.mul(osb[:, lam], ps[:], 1.0 / S2_SCALE)
                nc.gpsimd.dma_start(
                    out=out_d[img, og * 128:(og + 1) * 128],
                    in_=osb[:], accum_op=ADD)

        # software-pipelined emission: conv(k+1 block) interleaves with
        # ffn(k block) so PE never drains during the stats round-trip
        for step in range(BL_ + STAT_BLK):
            if step < BL_:
                conv_phase(step)
                if step % STAT_BLK == STAT_BLK - 1 or step == BL_ - 1:
                    stats_phase(step // STAT_BLK)
            j = step - STAT_BLK
            if 0 <= j < BL_:
                ffn_phase(j)
    nc.compile()
    return nc


# ---------------------------------------------------------------------------
# host side
# ---------------------------------------------------------------------------

SD_SCALE = 32.0     # conv diag weights scaled by this in fp8
S1_SCALE = None     # set per-run (pow2)
S2_SCALE = None


def _pow2_scale(absmax, target=200.0):
    if absmax <= 0:
        return 1.0
    return float(2.0 ** np.floor(np.log2(target / absmax)))


def _fold_host(inputs):
    global S1_SCALE, S2_SCALE
    f32 = np.float32
    fp8 = ml_dtypes.float8_e4m3
    dw_w = np.asarray(inputs["dw_w"], f32)
    dw_b = np.asarray(inputs["dw_b"], f32)
    norm_w = np.asarray(inputs["norm_w"], f32)
    norm_b = np.asarray(inputs["norm_b"], f32)
    w1 = np.asarray(inputs["w1"], f32)
    b1 = np.asarray(inputs["b1"], f32)
    w2 = np.asarray(inputs["w2"], f32)
    b2 = np.asarray(inputs["b2"], f32)
    gamma = np.asarray(inputs["gamma"], f32)
    fp_norm_w = np.asarray(inputs["fp_norm_w"], f32)
    fp_norm_b = np.asarray(inputs["fp_norm_b"], f32)
    fp_w = np.asarray(inputs["fp_w"], f32)
    fp_b = np.asarray(inputs["fp_b"], f32)
    fp_gamma = np.asarray(inputs["fp_gamma"], f32)

    W1 = norm_w[:, None] * w1
    c1 = norm_b @ w1 + b1
    W2f = w2 * gamma[None, :]
    c1out = b2 * gamma
    W2q = (fp_norm_w[:, None] * fp_w) * fp_gamma[None, :]
    c2 = (fp_norm_b @ fp_w + fp_b) * fp_gamma

    S1 = _pow2_scale(float(np.abs(W1).max()))
    S2 = _pow2_scale(max(float(np.abs(W2f).max()), float(np.abs(W2q).max())))
    S1_SCALE, S2_SCALE = S1, S2

    def q8(a):
        return np.clip(a, -240.0, 240.0).astype(fp8)

    # conv diag weights (scaled)
    diag2 = np.zeros((128, NG, NPAIR, 2, 128), f32)
    for g in range(NG):
        for pi, (ta, tb) in enumerate(PE_PAIRS):
            for j, (dy, dx) in enumerate((ta, tb)):
                wt = dw_w[g * 128:(g + 1) * 128, 0, dy + 3, dx + 3] * SD_SCALE
                diag2[np.arange(128), g, pi, j, np.arange(128)] = wt
    diag1 = np.zeros((128, NG, max(NSING, 1), 128), f32)
    for g in range(NG):
        for si, (dy, dx) in enumerate(PE_SINGLES):
            wt = dw_w[g * 128:(g + 1) * 128, 0, dy + 3, dx + 3] * SD_SCALE
            diag1[np.arange(128), g, si, np.arange(128)] = wt
    dwtap = np.zeros((128, NG, NDVE + NACT), f32)
    for g in range(NG):
        for k, (dy, dx) in enumerate(DVE_TAPS + ACT_TAPS):
            dwtap[:, g, k] = dw_w[g * 128:(g + 1) * 128, 0, dy + 3, dx + 3]
    dwb = np.zeros((128, NG), f32)
    for g in range(NG):
        dwb[:, g] = dw_b[g * 128:(g + 1) * 128]

    w1t = np.zeros((128, NG, FG, 128), f32)
    for cg in range(NG):
        for fg in range(FG):
            w1t[:, cg, fg, :] = W1[cg * 128:(cg + 1) * 128,
                                   fg * 128:(fg + 1) * 128] * S1
    w2ft = np.zeros((128, FG, NG, 128), f32)
    for fg in range(FG):
        for og in range(NG):
            w2ft[:, fg, og, :] = W2f[fg * 128:(fg + 1) * 128,
                                     og * 128:(og + 1) * 128] * S2
    w2qt = np.zeros((128, NG, NG, 128), f32)
    for cg in range(NG):
        for og in range(NG):
            w2qt[:, cg, og, :] = W2q[cg * 128:(cg + 1) * 128,
                                     og * 128:(og + 1) * 128] * S2

    slow = not (np.all(c1 == 0) and np.all(c1out == 0) and np.all(c2 == 0))
    extra = {}
    if slow:
        bf = ml_dtypes.bfloat16
        c1t = np.zeros((1, FG, 128), f32)
        for fg in range(FG):
            c1t[0, fg] = c1[fg * 128:(fg + 1) * 128] * S1
        ct = np.zeros((1, NG, 2, 128), f32)
        for og in range(NG):
            ct[0, og, 0] = c1out[og * 128:(og + 1) * 128] * S2
            ct[0, og, 1] = c2[og * 128:(og + 1) * 128] * S2
        extra = dict(c1t=c1t.astype(bf), ct=ct.astype(bf))

    return dict(
        diag2=q8(diag2), diag1=q8(diag1),
        w1t=q8(w1t), w2ft=q8(w2ft), w2qt=q8(w2qt),
        dwtap=dwtap, dwb=dwb, **extra,
    ), slow


def _masks_host(idx1, idx2, Bn):
    m2 = np.zeros((Bn, N), np.float32)
    np.put_along_axis(m2, np.asarray(idx2, np.int64), 1.0, axis=1)
    m1 = np.zeros((Bn, N), np.float32)
    np.put_along_axis(m1, np.asarray(idx1, np.int64), 1.0, axis=1)
    m1 = m1 * (1.0 - m2)  # reference scatter order: idx2 wins collisions
    return m1, m2


LAST_RESULT = None


def kernel(**inputs):
    global LAST_RESULT
    x = np.ascontiguousarray(np.asarray(inputs["x"], np.float32))
    Bn = x.shape[0]
    bl = Bn // N_CORES
    assert Bn % N_CORES == 0

    folded, slow = _fold_host(inputs)
    m1, m2 = _masks_host(inputs["idx1"], inputs["idx2"], Bn)
    # [B, branch, {C*m, m}, N]
    mrows = np.stack([np.stack([m1 * C, m1], 1),
                      np.stack([m2 * C, m2], 1)], 1).astype(ml_dtypes.bfloat16)

    nc = build_bass(bl, slow)

    in_maps = []
    for c in range(N_CORES):
        sl = slice(c * bl, (c + 1) * bl)
        in_maps.append(dict(
            x=x[sl],
            mrows=np.ascontiguousarray(mrows[sl]),
            **folded,
        ))

    trace = bool(int(os.environ.get("BASS_KERNEL_TRACE", "0")))
    res = run_bass_kernel_spmd(nc, in_maps, list(range(N_CORES)), trace=trace)
    LAST_RESULT = res
    out = np.concatenate([res.results[c]["out"] for c in range(N_CORES)], axis=0)
    return out


# revision 50
# speedup vs baseline: 1.0047x; 1.0047x over previous
"""Trainium2 Bass kernel for nn_AdaConvNeXt (moe_routing).

Data-parallel over batch (16 images/core).  Major design points vs the
previous version:
  - All FFN matmuls in fp8e4 with DoubleRow (K=256 per instruction):
    W1 (2 MMs per fg/half), W2f (6 MMs), W2q (2 MMs).
  - Routing masks are folded into the LayerNorm scale rows on the stats
    partitions (im = istd*m, nm = -mu*istd*m), so z1 = y*im1+nm1 and
    z2 = y*im2+nm2 are the *pre-masked* branch inputs.  Because gelu(0)=0
    and the biases are zero (fast path), both branch outputs accumulate
    into a single PSUM group: s12 = W2f^T gelu(W1^T z1) + W2q^T z2.
    Nonzero biases are handled by extra K=1 rank-1 matmuls (slow path).
  - Depthwise 7x7 conv split across engines with zero-padded halo tiles
    (row stride 48): PE gets fp8 DoubleRow tap *pairs* ((dy,dy+1), same
    dx) via hand-built overlapping access patterns; DVE gets fused
    STT multiply-adds for even-dx taps; ACT computes shifted products
    (alignment-immune) that GpSimd accumulates.
  - LN stats via ones-matmuls (both halves share one PSUM bank via
    tile_position), batched row math over 4 images.
"""

import os
import numpy as np
import ml_dtypes

import concourse.bass as bass
import concourse.bacc as bacc
import concourse.mybir as mybir
import concourse.tile as tile
from concourse.bass_utils import run_bass_kernel_spmd

BF16 = mybir.dt.bfloat16
FP8 = mybir.dt.float8e4
F32 = mybir.dt.float32
ADD = mybir.AluOpType.add
SUB = mybir.AluOpType.subtract
MULT = mybir.AluOpType.mult
AF = mybir.ActivationFunctionType
DR = mybir.MatmulPerfMode.DoubleRow

N_CORES = 8
B, C, H, W = 128, 384, 28, 28
N = H * W          # 784
BL = B // N_CORES  # 16 images per core
NG = C // 128      # 3 channel groups
FG = (4 * C) // 128  # 12 ffn groups
HALF = N // 2      # 392 = one PSUM bank of f32
EPS = 1e-6
STAT_BLK = 4       # images per batched-stats block (partitions 0,32,64,96)

# halo layout: row r = R0 + (h+dy), col = C0 + (w+dx), row stride HS
HS = 48
HR = 34
R0, C0 = 3, 4
NSLOT = 2          # x halo tile slots (double buffering)

# --- tap split (tunable) -----------------------------------------------------
# PE: dy-pairs (-3,-2),(-1,0),(1,2) per listed dx column
PE_PAIRS = [((a, dx), (a + 1, dx)) for dx in (-3, -1, 1, 3, 0, -2)
            for a in (-3, -1, 1)]
PE_SINGLES = []
# elementwise leftovers: dy=3 row + column dx=2
DVE_TAPS = [(3, 0), (3, -2), (-3, 2), (-2, 2), (-1, 2), (0, 2)]
_ASSIGNED = {t for p in PE_PAIRS for t in p} | set(PE_SINGLES) | set(DVE_TAPS)
ACT_TAPS = [(dy, dx) for dy in range(-3, 4) for dx in range(-3, 4)
            if (dy, dx) not in _ASSIGNED]
NPAIR = len(PE_PAIRS)
NSING = len(PE_SINGLES)
NDVE = len(DVE_TAPS)
NACT = len(ACT_TAPS)
assert 2 * NPAIR + NSING + NDVE + NACT == 49
DVE_INIT = (3, 0)  # via full-window halo product (init covers full acc)


def build_bass(BL_, slow_bias):
    nc = bacc.Bacc(None, target_bir_lowering=False, debug=False)

    x_d = nc.declare_dram_parameter("x", [BL_, C, H, W], F32, isOutput=False)
    # per image: [branch, {C*m, m}, N]
    mrows_d = nc.declare_dram_parameter("mrows", [BL_, 2, 2, N], BF16,
                                        isOutput=False)
    diag2_d = nc.declare_dram_parameter("diag2", [128, NG, NPAIR, 2, 128], FP8,
                                        isOutput=False)
    diag1_d = nc.declare_dram_parameter("diag1", [128, NG, max(NSING, 1), 128],
                                        FP8, isOutput=False)
    w1t_d = nc.declare_dram_parameter("w1t", [128, NG, FG, 128], FP8, isOutput=False)
    w2ft_d = nc.declare_dram_parameter("w2ft", [128, FG, NG, 128], FP8, isOutput=False)
    w2qt_d = nc.declare_dram_parameter("w2qt", [128, NG, NG, 128], FP8, isOutput=False)
    dwtap_d = nc.declare_dram_parameter("dwtap", [128, NG, NDVE + NACT], F32,
                                        isOutput=False)
    dwb_d = nc.declare_dram_parameter("dwb", [128, NG], F32, isOutput=False)
    if slow_bias:
        # c1*S1 per fg block; (c1out*S2, c2*S2) per og block
        c1t_d = nc.declare_dram_parameter("c1t", [1, FG, 128], BF16, isOutput=False)
        ct_d = nc.declare_dram_parameter("ct", [1, NG, 2, 128], BF16, isOutput=False)
    out_d = nc.declare_dram_parameter("out", [BL_, C, H, W], F32, isOutput=True)

    from contextlib import ExitStack
    with ExitStack() as es:
        tc = es.enter_context(tile.TileContext(nc))
        pool = lambda name, bufs, **kw: es.enter_context(
            tc.tile_pool(name=name, bufs=bufs, **kw))
        cpool = pool("consts", 1)
        acc_pool = pool("acc", 2)
        tmp_pool = pool("tmpp", 2)
        y_pool = pool("ybuf", STAT_BLK + 2)
        ysq_pool = pool("ysq", 1)
        t_pool = pool("tbuf", 2)
        z_pool = pool("zbuf", 2)
        g_pool = pool("gbuf", 2)
        bc_pool = pool("bcast", 2)
        o_pool = pool("obuf", 2)
        rows_pool = pool("rows", 1)
        dram_pool = pool("dscratch", 2 * STAT_BLK, space=bass.MemorySpace.DRAM)
        pyc_pool = pool("pyc", 2, space=bass.MemorySpace.PSUM)
        ph_pool = pool("ph", 2, space=bass.MemorySpace.PSUM)
        ps_pool = pool("ps", 2, space=bass.MemorySpace.PSUM)

        # ---- constants ----
        diag2_sb = cpool.tile([128, NG, NPAIR, 2, 128], FP8)
        nc.sync.dma_start(diag2_sb[:], diag2_d[:])
        diag1_sb = cpool.tile([128, NG, max(NSING, 1), 128], FP8)
        nc.sync.dma_start(diag1_sb[:], diag1_d[:])
        w1t_sb = cpool.tile([128, NG, FG, 128], FP8)
        nc.sync.dma_start(w1t_sb[:], w1t_d[:])
        w2ft_sb = cpool.tile([128, FG, NG, 128], FP8)
        nc.sync.dma_start(w2ft_sb[:], w2ft_d[:])
        w2qt_sb = cpool.tile([128, NG, NG, 128], FP8)
        nc.sync.dma_start(w2qt_sb[:], w2qt_d[:])
        dwtap_sb = cpool.tile([128, NG, NDVE + NACT], F32)
        nc.sync.dma_start(dwtap_sb[:], dwtap_d[:])
        dwb_sb = cpool.tile([128, NG], F32)
        nc.sync.dma_start(dwb_sb[:], dwb_d[:])
        if slow_bias:
            c1t_sb = cpool.tile([1, FG, 128], BF16)
            nc.sync.dma_start(c1t_sb[:], c1t_d[:])
            ct_sb = cpool.tile([1, NG, 2, 128], BF16)
            nc.sync.dma_start(ct_sb[:], ct_d[:])

        ones_col = cpool.tile([128, 1], BF16)
        nc.vector.memset(ones_col[:], 1.0)
        eps_col = cpool.tile([97, 1], F32)
        nc.vector.memset(eps_col[:], float(C) * float(C) * EPS)

        # persistent slotted halo tiles, borders zeroed once
        xf8 = cpool.tile([128, NSLOT, NG, HR, HS], FP8)
        xbf = cpool.tile([128, NSLOT, NG, HR, HS], BF16)
        for s in range(NSLOT):
            nc.gpsimd.memset(xf8[:, s], 0.0)
            nc.vector.memset(xbf[:, s], 0.0)

        def bcast3(tile_ap, n_inner):
            """[128, N]-tile AP broadcast to [128, NG, n_inner] via step-0 dim."""
            a = tile_ap
            return bass.AP(a.tensor, a.offset,
                           ap=[list(a.ap[0]), [0, NG], [1, n_inner]])

        def flat2(a, n_inner):
            """Contiguous free dims viewed as [2, n_inner]."""
            return bass.AP(a.tensor, a.offset,
                           ap=[list(a.ap[0]), [n_inner, 2], [1, n_inner]])

        def halo_win(xt, s, g, dy, dx, rows=H, cols=W, r_off=0):
            """AP over halo tile: [rows, cols] window shifted by (dy, dx)."""
            return xt[:, s, g,
                      R0 + r_off + dy: R0 + r_off + dy + rows,
                      C0 + dx: C0 + dx + cols]

        def pair_rhs(xt, s, g, dy0, dx, lam):
            """Hand-built overlapping AP [2, 14, 28] for DR tap pair."""
            base = xt[:, s, g]
            off = base.offset + (R0 + 14 * lam + dy0) * HS + (C0 + dx)
            return bass.AP(base.tensor, off,
                           ap=[list(base.ap[0]), [HS, 2], [HS, 14], [1, 28]])

        y_tiles = {}
        stat_dr = {}
        rows_t = {}
        mr1_t = {}

        def ensure_rows(blk):
            srow = rows_pool.tile([97, N], F32, tag="srow")
            qrow = rows_pool.tile([97, N], F32, tag="qrow")
            mrow = rows_pool.tile([97, 2, 2, N], BF16, tag="mrow")
            imgs = list(range(blk * STAT_BLK, min((blk + 1) * STAT_BLK, BL_)))
            for ii, img in enumerate(imgs):
                nc.sync.dma_start(out=mrow[32 * ii:32 * ii + 1],
                                  in_=mrows_d[img])
            rows_t[blk] = (srow, qrow, mrow, imgs)

        def conv_phase(img):
            blk, ii = divmod(img, STAT_BLK)
            if ii == 0:
                ensure_rows(blk)
            srow, qrow, _mrow, _imgs = rows_t[blk]
            s = img % NSLOT
            for g in range(NG):
                nc.gpsimd.dma_start(
                    out=xbf[:, s, g, R0:R0 + H, C0:C0 + W],
                    in_=x_d[img, g * 128:(g + 1) * 128])
                nc.gpsimd.dma_start(
                    out=xf8[:, s, g, R0:R0 + H, C0:C0 + W],
                    in_=x_d[img, g * 128:(g + 1) * 128])
                # residual prefill; branch outputs DMA-accumulate later
                nc.sync.dma_start(
                    out=out_d[img, g * 128:(g + 1) * 128],
                    in_=x_d[img, g * 128:(g + 1) * 128])

            # conv: elementwise part, two parallel accumulator chains
            acc = acc_pool.tile([128, NG, H, W], BF16, tag="accA")
            accB = acc_pool.tile([128, NG, H, W], BF16, tag="accB")
            k_init = DVE_TAPS.index(DVE_INIT)
            dy0, dx0 = DVE_INIT
            for g in range(NG):
                nc.vector.tensor_scalar(
                    out=acc[:, g], in0=halo_win(xbf, s, g, dy0, dx0),
                    scalar1=dwtap_sb[:, g, k_init:k_init + 1],
                    scalar2=dwb_sb[:, g:g + 1], op0=MULT, op1=ADD)
            for k, (dy, dx) in enumerate(DVE_TAPS):
                if (dy, dx) == DVE_INIT:
                    continue
                dtmp = tmp_pool.tile([128, NG, H, W], BF16, tag="dtmp")
                for g in range(NG):
                    nc.vector.tensor_scalar(
                        out=dtmp[:, g], in0=halo_win(xbf, s, g, dy, dx),
                        scalar1=dwtap_sb[:, g, k:k + 1],
                        scalar2=None, op0=MULT)
                nc.vector.tensor_tensor(
                    out=acc[:], in0=acc[:], in1=dtmp[:], op=ADD)
            for j, (dy, dx) in enumerate(ACT_TAPS):
                k = NDVE + j
                if j == 0:
                    tmp = accB  # first ACT product initializes the B chain
                else:
                    tmp = tmp_pool.tile([128, NG, H, W], BF16, tag="atmp")
                for g in range(NG):
                    nc.scalar.activation(
                        tmp[:, g], halo_win(xbf, s, g, dy, dx),
                        AF.Copy, scale=dwtap_sb[:, g, k:k + 1])
                if j > 0:
                    nc.vector.tensor_tensor(
                        out=accB[:], in0=accB[:], in1=tmp[:], op=ADD)

            # conv: PE fp8 DoubleRow pairs + singles
            y_bf = y_pool.tile([128, NG, 2, HALF], BF16)
            y_tiles[img] = y_bf
            for g in range(NG):
                pyc = pyc_pool.tile([128, 2, 512], F32)
                for pi, ((pdy0, pdx), _t1) in enumerate(PE_PAIRS):
                    for lam in range(2):
                        nc.tensor.matmul(
                            pyc[:, lam, 0:HALF],
                            diag2_sb[:, g, pi],
                            pair_rhs(xf8, s, g, pdy0, pdx, lam),
                            start=(pi == 0),
                            stop=(NSING == 0 and pi == NPAIR - 1),
                            perf_mode=DR, skip_group_check=True)
                for si, (dy, dx) in enumerate(PE_SINGLES):
                    for lam in range(2):
                        nc.tensor.matmul(
                            pyc[:, lam, 0:HALF],
                            diag1_sb[:, g, si],
                            halo_win(xf8, s, g, dy, dx, rows=14,
                                     r_off=14 * lam),
                            start=False, stop=(si == NSING - 1),
                            skip_group_check=True)
                # y = psum/S_dw + accA + accB
                nc.vector.scalar_tensor_tensor(
                    out=y_bf[:, g], in0=pyc[:, :, 0:HALF],
                    scalar=1.0 / SD_SCALE,
                    in1=flat2(acc[:, g], HALF),
                    op0=MULT, op1=ADD)
                nc.vector.tensor_tensor(
                    out=y_bf[:, g], in0=y_bf[:, g],
                    in1=flat2(accB[:, g], HALF), op=ADD)

            # LN stats
            ysq = ysq_pool.tile([128, NG, 2, HALF], BF16)
            nc.gpsimd.tensor_tensor(out=ysq[:], in0=y_bf[:], in1=y_bf[:],
                                    op=MULT)
            ps_base = 32 * ii
            for lam in range(2):
                pst = ph_pool.tile([128, HALF], F32, tag="ph")
                for g in range(NG):
                    nc.tensor.matmul(
                        pst[0:1, :], ones_col[:], y_bf[:, g, lam],
                        start=(g == 0), stop=(g == NG - 1),
                        skip_group_check=True)
                for g in range(NG):
                    nc.tensor.matmul(
                        pst[32:33, :], ones_col[:], ysq[:, g, lam],
                        start=(g == 0), stop=(g == NG - 1),
                        tile_position=(0, 32), skip_group_check=True)
                cs = slice(HALF * lam, HALF * lam + HALF)
                if lam == 0:
                    nc.vector.tensor_copy(srow[ps_base:ps_base + 1, cs],
                                          pst[0:1, :])
                    nc.vector.tensor_copy(qrow[ps_base:ps_base + 1, cs],
                                          pst[32:33, :])
                else:
                    nc.scalar.copy(srow[ps_base:ps_base + 1, cs],
                                   pst[0:1, :])
                    nc.scalar.copy(qrow[ps_base:ps_base + 1, cs],
                                   pst[32:33, :])

        def stats_phase(blk):
            srow, qrow, mrow, imgs = rows_t[blk]
            np_ = 32 * (len(imgs) - 1) + 1
            musq = rows_pool.tile([97, N], F32, tag="rw1")
            nc.vector.tensor_tensor(out=musq[:np_], in0=srow[:np_],
                                    in1=srow[:np_], op=MULT)
            veps = rows_pool.tile([97, N], F32, tag="rw2")
            nc.vector.scalar_tensor_tensor(
                out=veps[:np_], in0=qrow[:np_], scalar=float(C),
                in1=musq[:np_], op0=MULT, op1=SUB)
            sd = rows_pool.tile([97, N], F32, tag="rw1")
            nc.scalar.activation(sd[:np_], veps[:np_], AF.Sqrt,
                                 bias=eps_col[:np_])
            istd = rows_pool.tile([97, N], F32, tag="rw3")
            with nc.allow_low_precision(reason="LN istd approx is plenty"):
                nc.vector.reciprocal_approx_fast(out=istd[:np_], in_=sd[:np_])
            mus = rows_pool.tile([97, N], F32, tag="rw2")
            nc.vector.scalar_tensor_tensor(
                out=mus[:np_], in0=srow[:np_], scalar=-1.0,
                in1=istd[:np_], op0=MULT, op1=MULT)
            imr = rows_pool.tile([97, 2, N], BF16, tag="rw4")
            nmr = rows_pool.tile([97, 2, N], BF16, tag="rw5")
            for br in range(2):
                nc.vector.tensor_tensor(
                    out=imr[:np_, br], in0=istd[:np_],
                    in1=mrow[:np_, br, 0], op=MULT)
                nc.vector.tensor_tensor(
                    out=nmr[:np_, br], in0=mus[:np_],
                    in1=mrow[:np_, br, 1], op=MULT)
            # stage rows to DRAM for partition-broadcast
            for ii, img in enumerate(imgs):
                ps_base = 32 * ii
                sc = dram_pool.tile([4, N], BF16, tag="sc", name=f"sc{img}")
                nc.sync.dma_start(out=sc[0:1], in_=imr[ps_base:ps_base + 1, 0])
                nc.sync.dma_start(out=sc[1:2], in_=nmr[ps_base:ps_base + 1, 0])
                nc.sync.dma_start(out=sc[2:3], in_=imr[ps_base:ps_base + 1, 1])
                nc.sync.dma_start(out=sc[3:4], in_=nmr[ps_base:ps_base + 1, 1])
                stat_dr[img] = sc

        def ffn_phase(img):
            y_bf = y_tiles.pop(img)
            sc = stat_dr.pop(img)
            bcs = []
            for r in range(4):
                bt = bc_pool.tile([128, N], BF16, tag=f"bc{r}")
                nc.sync.dma_start(
                    out=bt[:], in_=sc[r:r + 1].partition_broadcast(128))
                bcs.append(bt)
            im1b, nm1b, im2b, nm2b = bcs
            if slow_bias:
                mr1 = bc_pool.tile([1, 2, N], BF16, tag="mr1")
                nc.sync.dma_start(out=mr1[:], in_=mrows_d[img, :, 1])

            z12 = []
            for br, (imb, nmb) in enumerate(((im1b, nm1b), (im2b, nm2b))):
                tb = t_pool.tile([128, NG, 2, HALF], BF16, tag="tb")
                zb = z_pool.tile([128, NG, 2, HALF], FP8, tag=f"z{br}")
                for g in range(NG):
                    nc.vector.tensor_tensor(
                        out=tb[:, g], in0=y_bf[:, g],
                        in1=flat2(imb[:], HALF), op=MULT)
                    nc.vector.tensor_tensor(
                        out=zb[:, g], in0=tb[:, g],
                        in1=flat2(nmb[:], HALF), op=ADD)
                z12.append(zb)
            z1, z2 = z12

            g_sb = g_pool.tile([128, FG, 2, HALF], FP8)
            for fg in range(FG):
                for lam in range(2):
                    ph = ph_pool.tile([128, HALF], F32, tag="ph")
                    nc.tensor.matmul(
                        ph[:], w1t_sb[:, 0:2, fg], z1[:, 0:2, lam],
                        start=True, stop=False, perf_mode=DR)
                    nc.tensor.matmul(
                        ph[:], w1t_sb[:, 2, fg], z1[:, 2, lam],
                        start=False, stop=not slow_bias)
                    if slow_bias:
                        nc.tensor.matmul(
                            ph[:], c1t_sb[:, fg],
                            mr1[:, 0, HALF * lam:HALF * lam + HALF],
                            start=False, stop=True, skip_group_check=True)
                    nc.scalar.activation(
                        g_sb[:, fg, lam], ph[:], AF.Gelu,
                        scale=1.0 / S1_SCALE)

            for og in range(NG):
                osb = o_pool.tile([128, 2, HALF], F32)
                for lam in range(2):
                    ps = ps_pool.tile([128, HALF], F32)
                    for f2 in range(FG // 2):
                        nc.tensor.matmul(
                            ps[:], w2ft_sb[:, 2 * f2:2 * f2 + 2, og],
                            g_sb[:, 2 * f2:2 * f2 + 2, lam],
                            start=(f2 == 0), stop=False, perf_mode=DR)
                    nc.tensor.matmul(
                        ps[:], w2qt_sb[:, 0:2, og], z2[:, 0:2, lam],
                        start=False, stop=False, perf_mode=DR)
                    nc.tensor.matmul(
                        ps[:], w2qt_sb[:, 2, og], z2[:, 2, lam],
                        start=False, stop=not slow_bias)
                    if slow_bias:
                        nc.tensor.matmul(
                            ps[:], ct_sb[:, og, 0],
                            mr1[:, 0, HALF * lam:HALF * lam + HALF],
                            start=False, stop=False, skip_group_check=True)
                        nc.tensor.matmul(
                            ps[:], ct_sb[:, og, 1],
                            mr1[:, 1, HALF * lam:HALF * lam + HALF],
                            start=False, stop=True, skip_group_check=True)
                    nc.vector.tensor_scalar(
                        out=osb[:, lam], in0=ps[:],
                        scalar1=1.0 / S2_SCALE, scalar2=None, op0=MULT)
                nc.gpsimd.dma_start(
                    out=out_d[img, og * 128:(og + 1) * 128],
                    in_=osb[:], accum_op=ADD)

        # software-pipelined emission: conv(k+1 block) interleaves with
        # ffn(k block) so PE never drains during the stats round-trip
        for step in range(BL_ + STAT_BLK):
            if step < BL_:
                conv_phase(step)
                if step % STAT_BLK == STAT_BLK - 1 or step == BL_ - 1:
                    stats_phase(step // STAT_BLK)
            j = step - STAT_BLK
            if 0 <= j < BL_:
                ffn_phase(j)
    nc.compile()
    return nc


# ---------------------------------------------------------------------------
# host side
# ---------------------------------------------------------------------------

SD_SCALE = 32.0     # conv diag weights scaled by this in fp8
S1_SCALE = None     # set per-run (pow2)
S2_SCALE = None


def _pow2_scale(absmax, target=200.0):
    if absmax <= 0:
        return 1.0
    return float(2.0 ** np.floor(np.log2(target / absmax)))


def _fold_host(inputs):
    global S1_SCALE, S2_SCALE
    f32 = np.float32
    fp8 = ml_dtypes.float8_e4m3
    dw_w = np.asarray(inputs["dw_w"], f32)
    dw_b = np.asarray(inputs["dw_b"], f32)
    norm_w = np.asarray(inputs["norm_w"], f32)
    norm_b = np.asarray(inputs["norm_b"], f32)
    w1 = np.asarray(inputs["w1"], f32)
    b1 = np.asarray(inputs["b1"], f32)
    w2 = np.asarray(inputs["w2"], f32)
    b2 = np.asarray(inputs["b2"], f32)
    gamma = np.asarray(inputs["gamma"], f32)
    fp_norm_w = np.asarray(inputs["fp_norm_w"], f32)
    fp_norm_b = np.asarray(inputs["fp_norm_b"], f32)
    fp_w = np.asarray(inputs["fp_w"], f32)
    fp_b = np.asarray(inputs["fp_b"], f32)
    fp_gamma = np.asarray(inputs["fp_gamma"], f32)

    W1 = norm_w[:, None] * w1
    c1 = norm_b @ w1 + b1
    W2f = w2 * gamma[None, :]
    c1out = b2 * gamma
    W2q = (fp_norm_w[:, None] * fp_w) * fp_gamma[None, :]
    c2 = (fp_norm_b @ fp_w + fp_b) * fp_gamma

    S1 = _pow2_scale(float(np.abs(W1).max()))
    S2 = _pow2_scale(max(float(np.abs(W2f).max()), float(np.abs(W2q).max())))
    S1_SCALE, S2_SCALE = S1, S2

    def q8(a):
        return np.clip(a, -240.0, 240.0).astype(fp8)

    # conv diag weights (scaled)
    diag2 = np.zeros((128, NG, NPAIR, 2, 128), f32)
    for g in range(NG):
        for pi, (ta, tb) in enumerate(PE_PAIRS):
            for j, (dy, dx) in enumerate((ta, tb)):
                wt = dw_w[g * 128:(g + 1) * 128, 0, dy + 3, dx + 3] * SD_SCALE
                diag2[np.arange(128), g, pi, j, np.arange(128)] = wt
    diag1 = np.zeros((128, NG, max(NSING, 1), 128), f32)
    for g in range(NG):
        for si, (dy, dx) in enumerate(PE_SINGLES):
            wt = dw_w[g * 128:(g + 1) * 128, 0, dy + 3, dx + 3] * SD_SCALE
            diag1[np.arange(128), g, si, np.arange(128)] = wt
    dwtap = np.zeros((128, NG, NDVE + NACT), f32)
    for g in range(NG):
        for k, (dy, dx) in enumerate(DVE_TAPS + ACT_TAPS):
            dwtap[:, g, k] = dw_w[g * 128:(g + 1) * 128, 0, dy + 3, dx + 3]
    dwb = np.zeros((128, NG), f32)
    for g in range(NG):
        dwb[:, g] = dw_b[g * 128:(g + 1) * 128]

    w1t = np.zeros((128, NG, FG, 128), f32)
    for cg in range(NG):
        for fg in range(FG):
            w1t[:, cg, fg, :] = W1[cg * 128:(cg + 1) * 128,
                                   fg * 128:(fg + 1) * 128] * S1
    w2ft = np.zeros((128, FG, NG, 128), f32)
    for fg in range(FG):
        for og in range(NG):
            w2ft[:, fg, og, :] = W2f[fg * 128:(fg + 1) * 128,
                                     og * 128:(og + 1) * 128] * S2
    w2qt = np.zeros((128, NG, NG, 128), f32)
    for cg in range(NG):
        for og in range(NG):
            w2qt[:, cg, og, :] = W2q[cg * 128:(cg + 1) * 128,
                                     og * 128:(og + 1) * 128] * S2

    slow = not (np.all(c1 == 0) and np.all(c1out == 0) and np.all(c2 == 0))
    extra = {}
    if slow:
        bf = ml_dtypes.bfloat16
        c1t = np.zeros((1, FG, 128), f32)
        for fg in range(FG):
            c1t[0, fg] = c1[fg * 128:(fg + 1) * 128] * S1
        ct = np.zeros((1, NG, 2, 128), f32)
        for og in range(NG):
            ct[0, og, 0] = c1out[og * 128:(og + 1) * 128] * S2
            ct[0, og, 1] = c2[og * 128:(og + 1) * 128] * S2
        extra = dict(c1t=c1t.astype(bf), ct=ct.astype(bf))

    return dict(
        diag2=q8(diag2), diag1=q8(diag1),
        w1t=q8(w1t), w2ft=q8(w2ft), w2qt=q8(w2qt),
        dwtap=dwtap, dwb=dwb, **extra,
    ), slow


def _masks_host(idx1, idx2, Bn):
    m2 = np.zeros((Bn, N), np.float32)
    np.put_along_axis(m2, np.asarray(idx2, np.int64), 1.0, axis=1)
    m1 = np.zeros((Bn, N), np.float32)
    np.put_along_axis(m1, np.asarray(idx1, np.int64), 1.0, axis=1)
    m1 = m1 * (1.0 - m2)  # reference scatter order: idx2 wins collisions
    return m1, m2


LAST_RESULT = None


def kernel(**inputs):
    global LAST_RESULT
    x = np.ascontiguousarray(np.asarray(inputs["x"], np.float32))
    Bn = x.shape[0]
    bl = Bn // N_CORES
    assert Bn % N_CORES == 0

    folded, slow = _fold_host(inputs)
    m1, m2 = _masks_host(inputs["idx1"], inputs["idx2"], Bn)
    # [B, branch, {C*m, m}, N]
    mrows = np.stack([np.stack([m1 * C, m1], 1),
                      np.stack([m2 * C, m2], 1)], 1).astype(ml_dtypes.bfloat16)

    nc = build_bass(bl, slow)

    in_maps = []
    for c in range(N_CORES):
        sl = slice(c * bl, (c + 1) * bl)
        in_maps.append(dict(
            x=x[sl],
            mrows=np.ascontiguousarray(mrows[sl]),
            **folded,
        ))

    trace = bool(int(os.environ.get("BASS_KERNEL_TRACE", "0")))
    res = run_bass_kernel_spmd(nc, in_maps, list(range(N_CORES)), trace=trace)
    LAST_RESULT = res
    out = np.concatenate([res.results[c]["out"] for c in range(N_CORES)], axis=0)
    return out


# revision 51
# speedup vs baseline: 1.0666x; 1.0616x over previous
"""Trainium2 Bass kernel for nn_AdaConvNeXt (moe_routing).

Data-parallel over batch (16 images/core).  Major design points vs the
previous version:
  - All FFN matmuls in fp8e4 with DoubleRow (K=256 per instruction):
    W1 (2 MMs per fg/half), W2f (6 MMs), W2q (2 MMs).
  - Routing masks are folded into the LayerNorm scale rows on the stats
    partitions (im = istd*m, nm = -mu*istd*m), so z1 = y*im1+nm1 and
    z2 = y*im2+nm2 are the *pre-masked* branch inputs.  Because gelu(0)=0
    and the biases are zero (fast path), both branch outputs accumulate
    into a single PSUM group: s12 = W2f^T gelu(W1^T z1) + W2q^T z2.
    Nonzero biases are handled by extra K=1 rank-1 matmuls (slow path).
  - Depthwise 7x7 conv split across engines with zero-padded halo tiles
    (row stride 48): PE gets fp8 DoubleRow tap *pairs* ((dy,dy+1), same
    dx) via hand-built overlapping access patterns; DVE gets fused
    STT multiply-adds for even-dx taps; ACT computes shifted products
    (alignment-immune) that GpSimd accumulates.
  - LN stats via ones-matmuls (both halves share one PSUM bank via
    tile_position), batched row math over 4 images.
"""

import os
import numpy as np
import ml_dtypes

import concourse.bass as bass
import concourse.bacc as bacc
import concourse.mybir as mybir
import concourse.tile as tile
from concourse.bass_utils import run_bass_kernel_spmd

BF16 = mybir.dt.bfloat16
FP8 = mybir.dt.float8e4
F32 = mybir.dt.float32
ADD = mybir.AluOpType.add
SUB = mybir.AluOpType.subtract
MULT = mybir.AluOpType.mult
AF = mybir.ActivationFunctionType
DR = mybir.MatmulPerfMode.DoubleRow

N_CORES = 8
B, C, H, W = 128, 384, 28, 28
N = H * W          # 784
BL = B // N_CORES  # 16 images per core
NG = C // 128      # 3 channel groups
FG = (4 * C) // 128  # 12 ffn groups
HALF = N // 2      # 392 = one PSUM bank of f32
EPS = 1e-6
STAT_BLK = 4       # images per batched-stats block (partitions 0,32,64,96)

# halo layout: row r = R0 + (h+dy), col = C0 + (w+dx), row stride HS
HS = 48
HR = 34
R0, C0 = 3, 4
NSLOT = 2          # x halo tile slots (double buffering)

# --- tap split (tunable) -----------------------------------------------------
# PE: dy-pairs (-3,-2),(-1,0),(1,2) per listed dx column
PE_PAIRS = [((a, dx), (a + 1, dx)) for dx in (-3, -1, 1, 3, 0, -2)
            for a in (-3, -1, 1)]
PE_SINGLES = []
# elementwise leftovers: dy=3 row + column dx=2
DVE_TAPS = [(3, 0), (3, -2), (-3, 2), (-2, 2), (-1, 2), (0, 2)]
_ASSIGNED = {t for p in PE_PAIRS for t in p} | set(PE_SINGLES) | set(DVE_TAPS)
ACT_TAPS = [(dy, dx) for dy in range(-3, 4) for dx in range(-3, 4)
            if (dy, dx) not in _ASSIGNED]
NPAIR = len(PE_PAIRS)
NSING = len(PE_SINGLES)
NDVE = len(DVE_TAPS)
NACT = len(ACT_TAPS)
assert 2 * NPAIR + NSING + NDVE + NACT == 49
DVE_INIT = (3, 0)  # via full-window halo product (init covers full acc)


def build_bass(BL_, slow_bias):
    nc = bacc.Bacc(None, target_bir_lowering=False, debug=False)

    x_d = nc.declare_dram_parameter("x", [BL_, C, H, W], F32, isOutput=False)
    # per image: [branch, {C*m, m}, N]
    mrows_d = nc.declare_dram_parameter("mrows", [BL_, 2, 2, N], BF16,
                                        isOutput=False)
    diag2_d = nc.declare_dram_parameter("diag2", [128, NG, NPAIR, 2, 128], FP8,
                                        isOutput=False)
    diag1_d = nc.declare_dram_parameter("diag1", [128, NG, max(NSING, 1), 128],
                                        FP8, isOutput=False)
    w1t_d = nc.declare_dram_parameter("w1t", [128, NG, FG, 128], FP8, isOutput=False)
    w2ft_d = nc.declare_dram_parameter("w2ft", [128, FG, NG, 128], FP8, isOutput=False)
    w2qt_d = nc.declare_dram_parameter("w2qt", [128, NG, NG, 128], FP8, isOutput=False)
    dwtap_d = nc.declare_dram_parameter("dwtap", [128, NG, NDVE + NACT], F32,
                                        isOutput=False)
    dwb_d = nc.declare_dram_parameter("dwb", [128, NG], F32, isOutput=False)
    if slow_bias:
        # c1*S1 per fg block; (c1out*S2, c2*S2) per og block
        c1t_d = nc.declare_dram_parameter("c1t", [1, FG, 128], BF16, isOutput=False)
        ct_d = nc.declare_dram_parameter("ct", [1, NG, 2, 128], BF16, isOutput=False)
    out_d = nc.declare_dram_parameter("out", [BL_, C, H, W], F32, isOutput=True)

    from contextlib import ExitStack
    with ExitStack() as es:
        tc = es.enter_context(tile.TileContext(nc))
        pool = lambda name, bufs, **kw: es.enter_context(
            tc.tile_pool(name=name, bufs=bufs, **kw))
        cpool = pool("consts", 1)
        acc_pool = pool("acc", 2)
        tmp_pool = pool("tmpp", 2)
        y_pool = pool("ybuf", STAT_BLK + 2)
        ysq_pool = pool("ysq", 1)
        t_pool = pool("tbuf", 2)
        z_pool = pool("zbuf", 2)
        g_pool = pool("gbuf", 2)
        bc_pool = pool("bcast", 2)
        o_pool = pool("obuf", 2)
        rows_pool = pool("rows", 1)
        dram_pool = pool("dscratch", 2 * STAT_BLK, space=bass.MemorySpace.DRAM)
        pyc_pool = pool("pyc", 2, space=bass.MemorySpace.PSUM)
        ph_pool = pool("ph", 2, space=bass.MemorySpace.PSUM)
        ps_pool = pool("ps", 2, space=bass.MemorySpace.PSUM)

        # ---- constants ----
        diag2_sb = cpool.tile([128, NG, NPAIR, 2, 128], FP8)
        nc.sync.dma_start(diag2_sb[:], diag2_d[:])
        diag1_sb = cpool.tile([128, NG, max(NSING, 1), 128], FP8)
        nc.sync.dma_start(diag1_sb[:], diag1_d[:])
        w1t_sb = cpool.tile([128, NG, FG, 128], FP8)
        nc.sync.dma_start(w1t_sb[:], w1t_d[:])
        w2ft_sb = cpool.tile([128, FG, NG, 128], FP8)
        nc.sync.dma_start(w2ft_sb[:], w2ft_d[:])
        w2qt_sb = cpool.tile([128, NG, NG, 128], FP8)
        nc.sync.dma_start(w2qt_sb[:], w2qt_d[:])
        dwtap_sb = cpool.tile([128, NG, NDVE + NACT], F32)
        nc.sync.dma_start(dwtap_sb[:], dwtap_d[:])
        dwb_sb = cpool.tile([128, NG], F32)
        nc.sync.dma_start(dwb_sb[:], dwb_d[:])
        if slow_bias:
            c1t_sb = cpool.tile([1, FG, 128], BF16)
            nc.sync.dma_start(c1t_sb[:], c1t_d[:])
            ct_sb = cpool.tile([1, NG, 2, 128], BF16)
            nc.sync.dma_start(ct_sb[:], ct_d[:])

        ones_col = cpool.tile([128, 1], BF16)
        nc.vector.memset(ones_col[:], 1.0)
        eps_col = cpool.tile([97, 1], F32)
        nc.vector.memset(eps_col[:], float(C) * float(C) * EPS)

        # persistent slotted halo tiles, borders zeroed once
        xf8 = cpool.tile([128, NSLOT, NG, HR, HS], FP8)
        xbf = cpool.tile([128, NSLOT, NG, HR, HS], BF16)
        for s in range(NSLOT):
            nc.gpsimd.memset(xf8[:, s], 0.0)
            nc.vector.memset(xbf[:, s], 0.0)

        def bcast3(tile_ap, n_inner):
            """[128, N]-tile AP broadcast to [128, NG, n_inner] via step-0 dim."""
            a = tile_ap
            return bass.AP(a.tensor, a.offset,
                           ap=[list(a.ap[0]), [0, NG], [1, n_inner]])

        def flat2(a, n_inner):
            """Contiguous free dims viewed as [2, n_inner]."""
            return bass.AP(a.tensor, a.offset,
                           ap=[list(a.ap[0]), [n_inner, 2], [1, n_inner]])

        def halo_win(xt, s, g, dy, dx, rows=H, cols=W, r_off=0):
            """AP over halo tile: [rows, cols] window shifted by (dy, dx)."""
            return xt[:, s, g,
                      R0 + r_off + dy: R0 + r_off + dy + rows,
                      C0 + dx: C0 + dx + cols]

        def pair_rhs(xt, s, g, dy0, dx, lam):
            """Hand-built overlapping AP [2, 14, 28] for DR tap pair."""
            base = xt[:, s, g]
            off = base.offset + (R0 + 14 * lam + dy0) * HS + (C0 + dx)
            return bass.AP(base.tensor, off,
                           ap=[list(base.ap[0]), [HS, 2], [HS, 14], [1, 28]])

        y_tiles = {}
        stat_dr = {}
        rows_t = {}
        mr1_t = {}

        def ensure_rows(blk):
            srow = rows_pool.tile([97, N], F32, tag="srow")
            qrow = rows_pool.tile([97, N], F32, tag="qrow")
            mrow = rows_pool.tile([97, 2, 2, N], BF16, tag="mrow")
            imgs = list(range(blk * STAT_BLK, min((blk + 1) * STAT_BLK, BL_)))
            for ii, img in enumerate(imgs):
                nc.sync.dma_start(out=mrow[32 * ii:32 * ii + 1],
                                  in_=mrows_d[img])
            rows_t[blk] = (srow, qrow, mrow, imgs)

        def conv_phase(img):
            blk, ii = divmod(img, STAT_BLK)
            if ii == 0:
                ensure_rows(blk)
            srow, qrow, _mrow, _imgs = rows_t[blk]
            s = img % NSLOT
            for g in range(NG):
                nc.gpsimd.dma_start(
                    out=xbf[:, s, g, R0:R0 + H, C0:C0 + W],
                    in_=x_d[img, g * 128:(g + 1) * 128])
                nc.gpsimd.dma_start(
                    out=xf8[:, s, g, R0:R0 + H, C0:C0 + W],
                    in_=x_d[img, g * 128:(g + 1) * 128])
                # residual prefill; branch outputs DMA-accumulate later
                nc.sync.dma_start(
                    out=out_d[img, g * 128:(g + 1) * 128],
                    in_=x_d[img, g * 128:(g + 1) * 128])

            # conv: elementwise part, two parallel accumulator chains
            acc = acc_pool.tile([128, NG, H, W], BF16, tag="accA")
            accB = acc_pool.tile([128, NG, H, W], BF16, tag="accB")
            k_init = DVE_TAPS.index(DVE_INIT)
            dy0, dx0 = DVE_INIT
            for g in range(NG):
                nc.vector.tensor_scalar(
                    out=acc[:, g], in0=halo_win(xbf, s, g, dy0, dx0),
                    scalar1=dwtap_sb[:, g, k_init:k_init + 1],
                    scalar2=dwb_sb[:, g:g + 1], op0=MULT, op1=ADD)
            for k, (dy, dx) in enumerate(DVE_TAPS):
                if (dy, dx) == DVE_INIT:
                    continue
                dtmp = tmp_pool.tile([128, NG, H, W], BF16, tag="dtmp")
                for g in range(NG):
                    nc.vector.tensor_scalar(
                        out=dtmp[:, g], in0=halo_win(xbf, s, g, dy, dx),
                        scalar1=dwtap_sb[:, g, k:k + 1],
                        scalar2=None, op0=MULT)
                nc.vector.tensor_tensor(
                    out=acc[:], in0=acc[:], in1=dtmp[:], op=ADD)
            for j, (dy, dx) in enumerate(ACT_TAPS):
                k = NDVE + j
                if j == 0:
                    tmp = accB  # first ACT product initializes the B chain
                else:
                    tmp = tmp_pool.tile([128, NG, H, W], BF16, tag="atmp")
                for g in range(NG):
                    nc.scalar.activation(
                        tmp[:, g], halo_win(xbf, s, g, dy, dx),
                        AF.Copy, scale=dwtap_sb[:, g, k:k + 1])
                if j > 0:
                    nc.vector.tensor_tensor(
                        out=accB[:], in0=accB[:], in1=tmp[:], op=ADD)

            # conv: PE fp8 DoubleRow pairs + singles
            y_bf = y_pool.tile([128, NG, 2, HALF], BF16)
            y_tiles[img] = y_bf
            for g in range(NG):
                pyc = pyc_pool.tile([128, 2, 512], F32)
                for pi, ((pdy0, pdx), _t1) in enumerate(PE_PAIRS):
                    for lam in range(2):
                        nc.tensor.matmul(
                            pyc[:, lam, 0:HALF],
                            diag2_sb[:, g, pi],
                            pair_rhs(xf8, s, g, pdy0, pdx, lam),
                            start=(pi == 0),
                            stop=(NSING == 0 and pi == NPAIR - 1),
                            perf_mode=DR, skip_group_check=True)
                for si, (dy, dx) in enumerate(PE_SINGLES):
                    for lam in range(2):
                        nc.tensor.matmul(
                            pyc[:, lam, 0:HALF],
                            diag1_sb[:, g, si],
                            halo_win(xf8, s, g, dy, dx, rows=14,
                                     r_off=14 * lam),
                            start=False, stop=(si == NSING - 1),
                            skip_group_check=True)
                # y = psum/S_dw + accA + accB
                nc.vector.scalar_tensor_tensor(
                    out=y_bf[:, g], in0=pyc[:, :, 0:HALF],
                    scalar=1.0 / SD_SCALE,
                    in1=flat2(acc[:, g], HALF),
                    op0=MULT, op1=ADD)
                nc.vector.tensor_tensor(
                    out=y_bf[:, g], in0=y_bf[:, g],
                    in1=flat2(accB[:, g], HALF), op=ADD)

            # LN stats
            ysq = ysq_pool.tile([128, NG, 2, HALF], BF16)
            nc.vector.tensor_tensor(out=ysq[:], in0=y_bf[:], in1=y_bf[:],
                                    op=MULT)
            ps_base = 32 * ii
            for lam in range(2):
                pst = ph_pool.tile([128, HALF], F32, tag="ph")
                for g in range(NG):
                    nc.tensor.matmul(
                        pst[0:1, :], ones_col[:], y_bf[:, g, lam],
                        start=(g == 0), stop=(g == NG - 1),
                        skip_group_check=True)
                for g in range(NG):
                    nc.tensor.matmul(
                        pst[32:33, :], ones_col[:], ysq[:, g, lam],
                        start=(g == 0), stop=(g == NG - 1),
                        tile_position=(0, 32), skip_group_check=True)
                cs = slice(HALF * lam, HALF * lam + HALF)
                if lam == 0:
                    nc.vector.tensor_copy(srow[ps_base:ps_base + 1, cs],
                                          pst[0:1, :])
                    nc.vector.tensor_copy(qrow[ps_base:ps_base + 1, cs],
                                          pst[32:33, :])
                else:
                    nc.scalar.copy(srow[ps_base:ps_base + 1, cs],
                                   pst[0:1, :])
                    nc.scalar.copy(qrow[ps_base:ps_base + 1, cs],
                                   pst[32:33, :])

        def stats_phase(blk):
            srow, qrow, mrow, imgs = rows_t[blk]
            np_ = 32 * (len(imgs) - 1) + 1
            musq = rows_pool.tile([97, N], F32, tag="rw1")
            nc.vector.tensor_tensor(out=musq[:np_], in0=srow[:np_],
                                    in1=srow[:np_], op=MULT)
            veps = rows_pool.tile([97, N], F32, tag="rw2")
            nc.vector.scalar_tensor_tensor(
                out=veps[:np_], in0=qrow[:np_], scalar=float(C),
                in1=musq[:np_], op0=MULT, op1=SUB)
            sd = rows_pool.tile([97, N], F32, tag="rw1")
            nc.scalar.activation(sd[:np_], veps[:np_], AF.Sqrt,
                                 bias=eps_col[:np_])
            istd = rows_pool.tile([97, N], F32, tag="rw3")
            with nc.allow_low_precision(reason="LN istd approx is plenty"):
                nc.vector.reciprocal_approx_fast(out=istd[:np_], in_=sd[:np_])
            mus = rows_pool.tile([97, N], F32, tag="rw2")
            nc.vector.scalar_tensor_tensor(
                out=mus[:np_], in0=srow[:np_], scalar=-1.0,
                in1=istd[:np_], op0=MULT, op1=MULT)
            imr = rows_pool.tile([97, 2, N], BF16, tag="rw4")
            nmr = rows_pool.tile([97, 2, N], BF16, tag="rw5")
            for br in range(2):
                nc.vector.tensor_tensor(
                    out=imr[:np_, br], in0=istd[:np_],
                    in1=mrow[:np_, br, 0], op=MULT)
                nc.vector.tensor_tensor(
                    out=nmr[:np_, br], in0=mus[:np_],
                    in1=mrow[:np_, br, 1], op=MULT)
            # stage rows to DRAM for partition-broadcast
            for ii, img in enumerate(imgs):
                ps_base = 32 * ii
                sc = dram_pool.tile([4, N], BF16, tag="sc", name=f"sc{img}")
                nc.sync.dma_start(out=sc[0:1], in_=imr[ps_base:ps_base + 1, 0])
                nc.sync.dma_start(out=sc[1:2], in_=nmr[ps_base:ps_base + 1, 0])
                nc.sync.dma_start(out=sc[2:3], in_=imr[ps_base:ps_base + 1, 1])
                nc.sync.dma_start(out=sc[3:4], in_=nmr[ps_base:ps_base + 1, 1])
                stat_dr[img] = sc

        def ffn_phase(img):
            y_bf = y_tiles.pop(img)
            sc = stat_dr.pop(img)
            bcs = []
            for r in range(4):
                bt = bc_pool.tile([128, N], BF16, tag=f"bc{r}")
                nc.sync.dma_start(
                    out=bt[:], in_=sc[r:r + 1].partition_broadcast(128))
                bcs.append(bt)
            im1b, nm1b, im2b, nm2b = bcs
            if slow_bias:
                mr1 = bc_pool.tile([1, 2, N], BF16, tag="mr1")
                nc.sync.dma_start(out=mr1[:], in_=mrows_d[img, :, 1])

            z12 = []
            for br, (imb, nmb) in enumerate(((im1b, nm1b), (im2b, nm2b))):
                tb = t_pool.tile([128, NG, 2, HALF], BF16, tag="tb")
                zb = z_pool.tile([128, NG, 2, HALF], FP8, tag=f"z{br}")
                for g in range(NG):
                    nc.vector.tensor_tensor(
                        out=tb[:, g], in0=y_bf[:, g],
                        in1=flat2(imb[:], HALF), op=MULT)
                    nc.vector.tensor_tensor(
                        out=zb[:, g], in0=tb[:, g],
                        in1=flat2(nmb[:], HALF), op=ADD)
                z12.append(zb)
            z1, z2 = z12

            g_sb = g_pool.tile([128, FG, 2, HALF], FP8)
            for fg in range(FG):
                for lam in range(2):
                    ph = ph_pool.tile([128, HALF], F32, tag="ph")
                    nc.tensor.matmul(
                        ph[:], w1t_sb[:, 0:2, fg], z1[:, 0:2, lam],
                        start=True, stop=False, perf_mode=DR)
                    nc.tensor.matmul(
                        ph[:], w1t_sb[:, 2, fg], z1[:, 2, lam],
                        start=False, stop=not slow_bias)
                    if slow_bias:
                        nc.tensor.matmul(
                            ph[:], c1t_sb[:, fg],
                            mr1[:, 0, HALF * lam:HALF * lam + HALF],
                            start=False, stop=True, skip_group_check=True)
                    nc.scalar.activation(
                        g_sb[:, fg, lam], ph[:], AF.Gelu,
                        scale=1.0 / S1_SCALE)

            for og in range(NG):
                osb = o_pool.tile([128, 2, HALF], F32)
                for lam in range(2):
                    ps = ps_pool.tile([128, HALF], F32)
                    for f2 in range(FG // 2):
                        nc.tensor.matmul(
                            ps[:], w2ft_sb[:, 2 * f2:2 * f2 + 2, og],
                            g_sb[:, 2 * f2:2 * f2 + 2, lam],
                            start=(f2 == 0), stop=False, perf_mode=DR)
                    nc.tensor.matmul(
                        ps[:], w2qt_sb[:, 0:2, og], z2[:, 0:2, lam],
                        start=False, stop=False, perf_mode=DR)
                    nc.tensor.matmul(
                        ps[:], w2qt_sb[:, 2, og], z2[:, 2, lam],
                        start=False, stop=not slow_bias)
                    if slow_bias:
                        nc.tensor.matmul(
                            ps[:], ct_sb[:, og, 0],
                            mr1[:, 0, HALF * lam:HALF * lam + HALF],
                            start=False, stop=False, skip_group_check=True)
                        nc.tensor.matmul(
                            ps[:], ct_sb[:, og, 1],
                            mr1[:, 1, HALF * lam:HALF * lam + HALF],
                            start=False, stop=True, skip_group_check=True)
                    nc.vector.tensor_scalar(
                        out=osb[:, lam], in0=ps[:],
                        scalar1=1.0 / S2_SCALE, scalar2=None, op0=MULT)
                nc.gpsimd.dma_start(
                    out=out_d[img, og * 128:(og + 1) * 128],
                    in_=osb[:], accum_op=ADD)

        # software-pipelined emission: conv(k+1 block) interleaves with
        # ffn(k block) so PE never drains during the stats round-trip
        for step in range(BL_ + STAT_BLK):
            if step < BL_:
                conv_phase(step)
                if step % STAT_BLK == STAT_BLK - 1 or step == BL_ - 1:
                    stats_phase(step // STAT_BLK)
            j = step - STAT_BLK
            if 0 <= j < BL_:
                ffn_phase(j)
    nc.compile()
    return nc


# ---------------------------------------------------------------------------
# host side
# ---------------------------------------------------------------------------

SD_SCALE = 32.0     # conv diag weights scaled by this in fp8
S1_SCALE = None     # set per-run (pow2)
S2_SCALE = None


def _pow2_scale(absmax, target=200.0):
    if absmax <= 0:
        return 1.0
    return float(2.0 ** np.floor(np.log2(target / absmax)))


def _fold_host(inputs):
    global S1_SCALE, S2_SCALE
    f32 = np.float32
    fp8 = ml_dtypes.float8_e4m3
    dw_w = np.asarray(inputs["dw_w"], f32)
    dw_b = np.asarray(inputs["dw_b"], f32)
    norm_w = np.asarray(inputs["norm_w"], f32)
    norm_b = np.asarray(inputs["norm_b"], f32)
    w1 = np.asarray(inputs["w1"], f32)
    b1 = np.asarray(inputs["b1"], f32)
    w2 = np.asarray(inputs["w2"], f32)
    b2 = np.asarray(inputs["b2"], f32)
    gamma = np.asarray(inputs["gamma"], f32)
    fp_norm_w = np.asarray(inputs["fp_norm_w"], f32)
    fp_norm_b = np.asarray(inputs["fp_norm_b"], f32)
    fp_w = np.asarray(inputs["fp_w"], f32)
    fp_b = np.asarray(inputs["fp_b"], f32)
    fp_gamma = np.asarray(inputs["fp_gamma"], f32)

    W1 = norm_w[:, None] * w1
    c1 = norm_b @ w1 + b1
    W2f = w2 * gamma[None, :]
    c1out = b2 * gamma
    W2q = (fp_norm_w[:, None] * fp_w) * fp_gamma[None, :]
    c2 = (fp_norm_b @ fp_w + fp_b) * fp_gamma

    S1 = _pow2_scale(float(np.abs(W1).max()))
    S2 = _pow2_scale(max(float(np.abs(W2f).max()), float(np.abs(W2q).max())))
    S1_SCALE, S2_SCALE = S1, S2

    def q8(a):
        return np.clip(a, -240.0, 240.0).astype(fp8)

    # conv diag weights (scaled)
    diag2 = np.zeros((128, NG, NPAIR, 2, 128), f32)
    for g in range(NG):
        for pi, (ta, tb) in enumerate(PE_PAIRS):
            for j, (dy, dx) in enumerate((ta, tb)):
                wt = dw_w[g * 128:(g + 1) * 128, 0, dy + 3, dx + 3] * SD_SCALE
                diag2[np.arange(128), g, pi, j, np.arange(128)] = wt
    diag1 = np.zeros((128, NG, max(NSING, 1), 128), f32)
    for g in range(NG):
        for si, (dy, dx) in enumerate(PE_SINGLES):
            wt = dw_w[g * 128:(g + 1) * 128, 0, dy + 3, dx + 3] * SD_SCALE
            diag1[np.arange(128), g, si, np.arange(128)] = wt
    dwtap = np.zeros((128, NG, NDVE + NACT), f32)
    for g in range(NG):
        for k, (dy, dx) in enumerate(DVE_TAPS + ACT_TAPS):
            dwtap[:, g, k] = dw_w[g * 128:(g + 1) * 128, 0, dy + 3, dx + 3]
    dwb = np.zeros((128, NG), f32)
    for g in range(NG):
        dwb[:, g] = dw_b[g * 128:(g + 1) * 128]

    w1t = np.zeros((128, NG, FG, 128), f32)
    for cg in range(NG):
        for fg in range(FG):
            w1t[:, cg, fg, :] = W1[cg * 128:(cg + 1) * 128,
                                   fg * 128:(fg + 1) * 128] * S1
    w2ft = np.zeros((128, FG, NG, 128), f32)
    for fg in range(FG):
        for og in range(NG):
            w2ft[:, fg, og, :] = W2f[fg * 128:(fg + 1) * 128,
                                     og * 128:(og + 1) * 128] * S2
    w2qt = np.zeros((128, NG, NG, 128), f32)
    for cg in range(NG):
        for og in range(NG):
            w2qt[:, cg, og, :] = W2q[cg * 128:(cg + 1) * 128,
                                     og * 128:(og + 1) * 128] * S2

    slow = not (np.all(c1 == 0) and np.all(c1out == 0) and np.all(c2 == 0))
    extra = {}
    if slow:
        bf = ml_dtypes.bfloat16
        c1t = np.zeros((1, FG, 128), f32)
        for fg in range(FG):
            c1t[0, fg] = c1[fg * 128:(fg + 1) * 128] * S1
        ct = np.zeros((1, NG, 2, 128), f32)
        for og in range(NG):
            ct[0, og, 0] = c1out[og * 128:(og + 1) * 128] * S2
            ct[0, og, 1] = c2[og * 128:(og + 1) * 128] * S2
        extra = dict(c1t=c1t.astype(bf), ct=ct.astype(bf))

    return dict(
        diag2=q8(diag2), diag1=q8(diag1),
        w1t=q8(w1t), w2ft=q8(w2ft), w2qt=q8(w2qt),
        dwtap=dwtap, dwb=dwb, **extra,
    ), slow


def _masks_host(idx1, idx2, Bn):
    m2 = np.zeros((Bn, N), np.float32)
    np.put_along_axis(m2, np.asarray(idx2, np.int64), 1.0, axis=1)
    m1 = np.zeros((Bn, N), np.float32)
    np.put_along_axis(m1, np.asarray(idx1, np.int64), 1.0, axis=1)
    m1 = m1 * (1.0 - m2)  # reference scatter order: idx2 wins collisions
    return m1, m2


LAST_RESULT = None


def kernel(**inputs):
    global LAST_RESULT
    x = np.ascontiguousarray(np.asarray(inputs["x"], np.float32))
    Bn = x.shape[0]
    bl = Bn // N_CORES
    assert Bn % N_CORES == 0

    folded, slow = _fold_host(inputs)
    m1, m2 = _masks_host(inputs["idx1"], inputs["idx2"], Bn)
    # [B, branch, {C*m, m}, N]
    mrows = np.stack([np.stack([m1 * C, m1], 1),
                      np.stack([m2 * C, m2], 1)], 1).astype(ml_dtypes.bfloat16)

    nc = build_bass(bl, slow)

    in_maps = []
    for c in range(N_CORES):
        sl = slice(c * bl, (c + 1) * bl)
        in_maps.append(dict(
            x=x[sl],
            mrows=np.ascontiguousarray(mrows[sl]),
            **folded,
        ))

    trace = bool(int(os.environ.get("BASS_KERNEL_TRACE", "0")))
    res = run_bass_kernel_spmd(nc, in_maps, list(range(N_CORES)), trace=trace)
    LAST_RESULT = res
    out = np.concatenate([res.results[c]["out"] for c in range(N_CORES)], axis=0)
    return out
